# revision 16
# baseline (speedup 1.0000x reference)
"""Distributed Trainium2 (Bass/Tile) kernel for nn_Attention_2D.

Pipeline (per batch element): 3x3 conv + BatchNorm (batch stats!) for
Q (from x), K, V (from y) -> linear projections -> multi-head attention
(scale = C**-0.5) -> output projection.

Sharding: data-parallel over batch B=8 across the 8 NeuronCores (one
image per core). The only cross-core dependency is the BatchNorm
mean/var over the whole batch -> tiny [128,{8,4}] AllReduces.

v2 schedule notes (the baseline staged scores through SBUF via VectorE
copies - 87us of DVE time - and ran conv_v before the projections,
pushing the 73us ScalarE exp stream to start ~50us in):
  - scores stay in PSUM ([128,1024] tiles, 2 banks each, 3 bufs) and
    ScalarE exps them PSUM->SBUF bf16 directly; no staging copies.
  - the exp stream is the attention bottleneck (64 x ~1.15us), so the
    emission order is built around starting it as early as possible and
    never starving it: conv_k, conv_q, CC1 (stats) covered by one
    quarter of conv_v, BN, q/k projections, then attention group
    (g=0,lh=0) begins; the remaining 3 quarters of conv_v + CC2 + the
    v projection are interleaved between score/exp units of that first
    block, whose attn@V matmuls are deferred (ptc tiles buffered) until
    v_sb lands. Later blocks run a lag-3 score->exp->attn@V pipeline.
  - BN rstd uses a DVE Newton iteration (bit-trick seed) instead of
    ScalarE Ln/Exp: the act-table loader thrashes sets otherwise (Ln
    and Exp resolve to different table sets -> 4 extra ~1.3us loads,
    two of them in front of the exp stream).
  - PSUM->SBUF copies (projections) and the output-proj bias add run on
    VectorE, keeping ScalarE exp-only.
  - optional: a fraction of exp tiles can run on VectorE via a
    Schraudolph bf16-bit-trick tensor_scalar (DVE_EXP_EVERY).
"""

import os

import numpy as np

B, L, C = 8, 1024, 256
H = 8
D = 32  # head dim
IMG = 32  # h = w = 32
PAD = 34  # padded image side
EPS = 1e-5
ATT_SCALE = float(C) ** -0.5  # 1/16

# Schraudolph exp in bf16-bits: exp(s*x) ~= bitcast_bf16(i16(x*SCH_A + SCH_B))
SCH_A = (128.0 / float(np.log(2.0))) * ATT_SCALE
SCH_B = 128.0 * (127.0 - 0.0450466)

_CACHE = {}
DEBUG = False
VARIANT = "full"  # "full" | "noattn" | "convonly" (phase timing builds)
SIM_NO_CC = False  # replace AllReduce with local DMA copy (TimelineSim only)
RSTD_MODE = "newton"  # "newton" (DVE) | "lnexp" (ScalarE tables)
# 0 = off; n>0: every nth exp tile on DVE (Schraudolph)
DVE_EXP_EVERY = int(os.environ.get("DVE_EXP_EVERY", "0"))
# fp8e4m3 convs with DoubleRow (2x PE throughput); weights are scaled by
# W8_SCALE host-side to clear the fp8 subnormal range - BatchNorm is
# per-channel scale-invariant, so nothing downstream changes.
CONV_FP8 = int(os.environ.get("CONV_FP8", "0"))
W8_SCALE = 64.0
PADW8 = 40  # fp8 padded image row width (ci stride 34*40 B, 16-aligned)


def _build_nc(repeat=1):
    import concourse.bacc as bacc
    import concourse.tile as tile
    from concourse import mybir

    f32 = mybir.dt.float32
    f32r = mybir.dt.float32r
    bf16 = mybir.dt.bfloat16
    i16 = mybir.dt.int16
    i32 = mybir.dt.int32
    fp8 = mybir.dt.float8e4
    AF = mybir.ActivationFunctionType
    ALU = mybir.AluOpType

    nc = bacc.Bacc(None, target_bir_lowering=False)
    nc.num_devices = 8

    # ---- DRAM parameters (host-prepped layouts) ----
    if CONV_FP8:
        xt = nc.declare_dram_parameter("xt", [C, 34 * PADW8], fp8, isOutput=False)
        yt = nc.declare_dram_parameter("yt", [C, 34 * PADW8], fp8, isOutput=False)
        # conv weights: [9(kpos), 2(ci), 2(co), 128, 128] fp8, x W8_SCALE
        wcq = nc.declare_dram_parameter("wcq", [9, 2, 2, 128, 128], fp8, isOutput=False)
        wck = nc.declare_dram_parameter("wck", [9, 2, 2, 128, 128], fp8, isOutput=False)
        wcv = nc.declare_dram_parameter("wcv", [9, 2, 2, 128, 128], fp8, isOutput=False)
    else:
        xt = nc.declare_dram_parameter("xt", [C, PAD * PAD], f32r, isOutput=False)
        yt = nc.declare_dram_parameter("yt", [C, PAD * PAD], f32r, isOutput=False)
        # conv weights: [9(kpos), 2(ci), 2(co), 128, 128]
        wcq = nc.declare_dram_parameter("wcq", [9, 2, 2, 128, 128], f32r, isOutput=False)
        wck = nc.declare_dram_parameter("wck", [9, 2, 2, 128, 128], f32r, isOutput=False)
        wcv = nc.declare_dram_parameter("wcv", [9, 2, 2, 128, 128], f32r, isOutput=False)
    # projection weights W.T tiled: [2(ci), 128, 256(co)]
    pq = nc.declare_dram_parameter("pq", [2, 128, C], f32r, isOutput=False)
    pk = nc.declare_dram_parameter("pk", [2, 128, C], f32r, isOutput=False)
    pv = nc.declare_dram_parameter("pv", [2, 128, C], f32r, isOutput=False)
    po = nc.declare_dram_parameter("po", [2, 128, C], f32r, isOutput=False)
    # gamma/beta pack [128, 12]: cols 0-5 gamma, 6-11 beta
    gb = nc.declare_dram_parameter("gb", [128, 12], f32, isOutput=False)
    bo = nc.declare_dram_parameter("bo", [128, 2], f32, isOutput=False)
    out = nc.declare_dram_parameter("out", [C, L], f32, isOutput=True)

    with tile.TileContext(nc) as tc:
        with tc.tile_pool(name="singles", bufs=1) as singles, \
             tc.tile_pool(name="stats", bufs=1) as statsp, \
             tc.tile_pool(name="bnst", bufs=4) as bnstp, \
             tc.tile_pool(name="rep", bufs=2) as repp, \
             tc.tile_pool(name="pt", bufs=20) as ptp, \
             tc.tile_pool(name="ps", bufs=2, space="PSUM") as psp, \
             tc.tile_pool(name="score_ps", bufs=3, space="PSUM") as scorep, \
             tc.tile_pool(name="dram", bufs=1, space="DRAM") as dramp:

            for _rep in range(repeat):
                # ---------- constants / small tiles ----------
                ones32 = singles.tile([128, 32], bf16)
                nc.vector.memset(ones32[:], 1.0)
                epst = singles.tile([128, 1], f32)
                nc.vector.memset(epst[:], EPS)
                magict = singles.tile([128, 6], f32)
                # f32 whose bits are 0x5f375a86 (Newton-rsqrt magic)
                nc.vector.memset(magict[:], 1.3212019791402893e19)
                gbt = singles.tile([128, 12], f32)
                nc.sync.dma_start(out=gbt[:], in_=gb[:])
                bot = singles.tile([128, 2], f32)
                nc.sync.dma_start(out=bot[:], in_=bo[:])

                # ---------- padded images + weights ----------
                cw_dt = fp8 if CONV_FP8 else f32r
                if CONV_FP8:
                    pad_x = singles.tile([128, 2, 34, PADW8], fp8)
                    pad_y = singles.tile([128, 2, 34, PADW8], fp8)
                    ytr = yt.rearrange("(c p) (r w) -> p c r w", p=128, w=PADW8)
                    xtr = xt.rearrange("(c p) (r w) -> p c r w", p=128, w=PADW8)
                else:
                    pad_x = singles.tile([128, 2, PAD, PAD], f32r)
                    pad_y = singles.tile([128, 2, PAD, PAD], f32r)
                    ytr = yt.rearrange("(c p) m -> p c m", p=128)
                    xtr = xt.rearrange("(c p) m -> p c m", p=128)
                wq_sb = singles.tile([128, 36 * 128], cw_dt)
                wk_sb = singles.tile([128, 36 * 128], cw_dt)
                wv_sb = singles.tile([128, 36 * 128], cw_dt)
                pq_sb = singles.tile([128, 2 * C], f32r)
                pk_sb = singles.tile([128, 2 * C], f32r)
                pv_sb = singles.tile([128, 2 * C], f32r)
                po_sb = singles.tile([128, 2 * C], f32r)

                wk4 = wk_sb[:].rearrange("p (a t f) -> p a t f", a=3, f=128)
                wq4 = wq_sb[:].rearrange("p (a t f) -> p a t f", a=3, f=128)
                wv4 = wv_sb[:].rearrange("p (a t f) -> p a t f", a=3, f=128)
                wckr = wck.rearrange("(a g) b c p f -> p a (g b c) f", a=3)
                wcqr = wcq.rearrange("(a g) b c p f -> p a (g b c) f", a=3)
                wcvr = wcv.rearrange("(a g) b c p f -> p a (g b c) f", a=3)
                # two HWDGE rings, FIFO each; emit in consumption order:
                # sync: pad_y -> wcq -> pk -> pv ; scalar: wck -> pad_x -> pq
                # -> wcv -> po.  conv_k needs pad_y(sync)+wck(scalar) ~4.5us.
                for ci in range(2):
                    nc.sync.dma_start(out=pad_y[:, ci], in_=ytr[:, ci])
                for a in range(3):
                    nc.scalar.dma_start(out=wk4[:, a], in_=wckr[:, a])
                for a in range(3):
                    nc.sync.dma_start(out=wq4[:, a], in_=wcqr[:, a])
                for ci in range(2):
                    nc.scalar.dma_start(out=pad_x[:, ci], in_=xtr[:, ci])
                for eng, psb, pdr in ((nc.sync, pk_sb, pk), (nc.scalar, pq_sb, pq),
                                      (nc.sync, pv_sb, pv)):
                    eng.dma_start(
                        out=psb[:].rearrange("p (t f) -> p t f", f=C),
                        in_=pdr.rearrange("t p f -> p t f"),
                    )
                for a in range(3):
                    nc.scalar.dma_start(out=wv4[:, a], in_=wcvr[:, a])
                nc.scalar.dma_start(
                    out=po_sb[:].rearrange("p (t f) -> p t f", f=C),
                    in_=po.rearrange("t p f -> p t f"),
                )

                # ---------- conv machinery ----------
                kraw = singles.tile([128, 2 * L], f32)
                vraw = singles.tile([128, 2 * L], f32)
                qraw = singles.tile([128, 2 * L], f32)
                st = statsp.tile([128, 12], f32)  # local (mean, m2) pairs

                def conv_quarter(pad_t, w_sb, raw, stat_base, co, half):
                    ps = psp.tile([128, 512], f32, tag="ps")
                    if CONV_FP8:
                        w8v = w_sb[:].rearrange(
                            "p (kp ci co f) -> p kp ci co f", kp=9, ci=2, co=2)
                        for kp in range(9):
                            ky, kx = kp // 3, kp % 3
                            lhsT = w8v[:, kp, :, co, :]
                            rhs = pad_t[:, :, ky + half * 16: ky + half * 16 + 16,
                                        kx: kx + 32]
                            nc.tensor.matmul(
                                ps[:], lhsT, rhs, start=(kp == 0), stop=(kp == 8),
                                perf_mode=mybir.MatmulPerfMode.DoubleRow)
                    else:
                        idx = 0
                        for kp in range(9):
                            ky, kx = kp // 3, kp % 3
                            for ci in range(2):
                                blk = (kp * 2 + ci) * 2 + co
                                lhsT = w_sb[:, blk * 128:(blk + 1) * 128]
                                rhs = pad_t[:, ci, ky + half * 16: ky + half * 16 + 16,
                                            kx: kx + 32]
                                nc.tensor.matmul(ps[:], lhsT, rhs,
                                                 start=(idx == 0), stop=(idx == 17))
                                idx += 1
                    nc.vector.tensor_copy(
                        out=raw[:, co * L + half * 512: co * L + (half + 1) * 512].bitcast(f32r),
                        in_=ps[:])
                    if half == 1:
                        k = stat_base + co
                        st6 = bnstp.tile([128, 2, 6], f32, tag="st6")
                        nc.vector.bn_stats(st6[:, 0, :], raw[:, co * L: co * L + 512])
                        nc.vector.bn_stats(st6[:, 1, :], raw[:, co * L + 512: co * L + 1024])
                        nc.vector.bn_aggr(st[:, 2 * k: 2 * k + 2], st6[:])
                        # m2 = mean^2 + var (in place on the var column)
                        nc.vector.scalar_tensor_tensor(
                            out=st[:, 2 * k + 1: 2 * k + 2],
                            in0=st[:, 2 * k: 2 * k + 1],
                            scalar=st[:, 2 * k: 2 * k + 1],
                            in1=st[:, 2 * k + 1: 2 * k + 2],
                            op0=ALU.mult, op1=ALU.add,
                        )

                def conv_full(pad_t, w_sb, raw, stat_base):
                    for co in range(2):
                        for half in range(2):
                            conv_quarter(pad_t, w_sb, raw, stat_base, co, half)

                conv_full(pad_y, wk_sb, kraw, 2)
                conv_full(pad_x, wq_sb, qraw, 0)

                # ---------- AllReduce #1: q+k stats ----------
                cc_in1 = dramp.tile([128, 8], f32)
                cc_out1 = dramp.tile([128, 8], f32)
                nc.sync.dma_start(out=cc_in1[:], in_=st[:, 0:8])
                if SIM_NO_CC:
                    nc.gpsimd.dma_start(out=cc_out1[:], in_=cc_in1[:])
                else:
                    nc.gpsimd.collective_compute(
                        "AllReduce", ALU.add,
                        replica_groups=[list(range(8))],
                        ins=[cc_in1[:].opt()], outs=[cc_out1[:].opt()],
                    )
                gstats = statsp.tile([128, 12], f32)
                nc.sync.dma_start(out=gstats[:, 0:8], in_=cc_out1[:])

                # first quarter of conv_v covers the CC1 latency
                if VARIANT == "full":
                    conv_quarter(pad_y, wv_sb, vraw, 4, 0, 0)
                else:
                    conv_full(pad_y, wv_sb, vraw, 4)

                # ---------- global scale/shift ----------
                var_t = statsp.tile([128, 6], f32)
                nwt_h = statsp.tile([128, 6], f32)
                nwt_y = statsp.tile([128, 6], f32)
                nwt_t = statsp.tile([128, 6], f32)
                scale_t = statsp.tile([128, 6], f32)
                shift_t = statsp.tile([128, 6], f32)

                def bn_post(k0, nk):
                    seg = gstats[:, 2 * k0: 2 * (k0 + nk)]
                    nc.vector.tensor_scalar_mul(seg, seg, 1.0 / 8.0)
                    g2 = seg.rearrange("p (k two) -> p k two", two=2)
                    gmean = g2[:, :, 0]
                    gm2 = g2[:, :, 1]
                    vt = var_t[:, k0: k0 + nk]
                    nc.vector.tensor_mul(vt, gmean, gmean)
                    nc.vector.tensor_sub(vt, gm2, vt)
                    if RSTD_MODE == "newton":
                        # rstd = rsqrt(var+eps): bit-trick seed + 2 Newton steps
                        nc.vector.tensor_scalar_add(vt, vt, EPS)
                        hv = nwt_h[:, k0: k0 + nk]
                        nc.vector.tensor_scalar_mul(hv, vt, 0.5)
                        yv = nwt_y[:, k0: k0 + nk]
                        nc.vector.tensor_scalar(
                            out=yv.bitcast(i32), in0=vt.bitcast(i32),
                            scalar1=1, scalar2=None, op0=ALU.logical_shift_right)
                        nc.vector.tensor_sub(
                            yv.bitcast(i32), magict[:, k0: k0 + nk].bitcast(i32),
                            yv.bitcast(i32))
                        tv = nwt_t[:, k0: k0 + nk]
                        for _ in range(2):
                            nc.vector.tensor_mul(tv, yv, yv)
                            nc.vector.tensor_mul(tv, tv, hv)
                            nc.vector.tensor_scalar(
                                out=tv, in0=tv, scalar1=-1.0, scalar2=1.5,
                                op0=ALU.mult, op1=ALU.add)
                            nc.vector.tensor_mul(yv, yv, tv)
                        vt = yv
                    else:
                        nc.scalar.activation(vt, vt, AF.Ln, bias=epst[:, 0:1], scale=1.0)
                        nc.scalar.activation(vt, vt, AF.Exp, scale=-0.5)
                    sc = scale_t[:, k0: k0 + nk]
                    sh = shift_t[:, k0: k0 + nk]
                    nc.vector.tensor_mul(sc, vt, gbt[:, k0: k0 + nk])
                    nc.vector.tensor_mul(sh, gmean, sc)
                    nc.vector.tensor_sub(sh, gbt[:, 6 + k0: 6 + k0 + nk], sh)

                def bn_apply(raw, base):
                    for ch in range(2):
                        k = base + ch
                        nc.vector.tensor_scalar(
                            out=raw[:, ch * L:(ch + 1) * L].bitcast(f32r),
                            in0=raw[:, ch * L:(ch + 1) * L],
                            scalar1=scale_t[:, k: k + 1],
                            scalar2=shift_t[:, k: k + 1],
                            op0=ALU.mult, op1=ALU.add,
                        )

                bn_post(0, 4)   # q, k
                bn_apply(qraw, 0)
                bn_apply(kraw, 2)

                if VARIANT == "convonly":
                    cc_in2 = dramp.tile([128, 4], f32)
                    cc_out2 = dramp.tile([128, 4], f32)
                    nc.sync.dma_start(out=cc_in2[:], in_=st[:, 8:12])
                    if SIM_NO_CC:
                        nc.gpsimd.dma_start(out=cc_out2[:], in_=cc_in2[:])
                    else:
                        nc.gpsimd.collective_compute(
                            "AllReduce", ALU.add,
                            replica_groups=[list(range(8))],
                            ins=[cc_in2[:].opt()], outs=[cc_out2[:].opt()],
                        )
                    nc.sync.dma_start(out=gstats[:, 8:12], in_=cc_out2[:])
                    bn_post(4, 2)
                    bn_apply(vraw, 4)
                    nc.sync.dma_start(
                        out=out.rearrange("(c p) l -> p c l", p=128),
                        in_=kraw[:].rearrange("p (c l) -> p c l", l=L))
                    continue

                # ---------- q/k projections -> transposed [c, L] ----------
                qT = singles.tile([128, 2 * L], f32)
                kT = singles.tile([128, 2 * L], f32)

                def proj_T(src_t, wsb, dst, co):
                    for lh in range(2):
                        ps = psp.tile([128, 512], f32, tag="ps")
                        for ci in range(2):
                            lhsT = wsb[:, ci * C + co * 128: ci * C + (co + 1) * 128]
                            rhs = src_t[:, ci * L + lh * 512: ci * L + (lh + 1) * 512].bitcast(f32r)
                            nc.tensor.matmul(ps[:], lhsT, rhs,
                                             start=(ci == 0), stop=(ci == 1))
                        nc.vector.tensor_copy(
                            out=dst[:, co * L + lh * 512: co * L + (lh + 1) * 512].bitcast(f32r),
                            in_=ps[:])

                for co in range(2):
                    proj_T(kraw, pk_sb, kT, co)
                    proj_T(qraw, pq_sb, qT, co)

                # ---------- v path helpers (emitted later, interleaved) ----
                v_sb = singles.tile([128, 8 * C], bf16)  # col = tc*256 + co

                def emit_cc2_bn_v():
                    cc_in2 = dramp.tile([128, 4], f32)
                    cc_out2 = dramp.tile([128, 4], f32)
                    nc.sync.dma_start(out=cc_in2[:], in_=st[:, 8:12])
                    if SIM_NO_CC:
                        nc.gpsimd.dma_start(out=cc_out2[:], in_=cc_in2[:])
                    else:
                        nc.gpsimd.collective_compute(
                            "AllReduce", ALU.add,
                            replica_groups=[list(range(8))],
                            ins=[cc_in2[:].opt()], outs=[cc_out2[:].opt()],
                        )
                    nc.sync.dma_start(out=gstats[:, 8:12], in_=cc_out2[:])
                    bn_post(4, 2)
                    bn_apply(vraw, 4)

                def emit_vproj():
                    for lt in range(8):
                        ps = psp.tile([128, C], f32, tag="ps")
                        for ci in range(2):
                            lhsT = vraw[:, ci * L + lt * 128: ci * L + (lt + 1) * 128].bitcast(f32r)
                            rhs = pv_sb[:, ci * C:(ci + 1) * C]
                            nc.tensor.matmul(ps[:], lhsT, rhs, start=(ci == 0), stop=(ci == 1))
                        nc.vector.tensor_copy(out=v_sb[:, lt * C:(lt + 1) * C], in_=ps[:])

                if VARIANT == "noattn":
                    emit_cc2_bn_v()
                    emit_vproj()
                    nc.sync.dma_start(
                        out=out.rearrange("(c p) l -> p c l", p=128),
                        in_=qT[:].rearrange("p (c l) -> p c l", l=L))
                    continue

                # ---------- attention ----------
                attn_oT = singles.tile([128, 2 * L], f32)  # col = g*1024 + l
                exp_ctr = [0]

                def sc_unit(g, lh, tc_i, jp):
                    score = scorep.tile([128, 1024], f32, tag="score")
                    for jj in range(2):
                        j = 2 * jp + jj
                        lhsT = kT[32 * j: 32 * j + 32,
                                  g * L + tc_i * 128: g * L + (tc_i + 1) * 128].bitcast(f32r)
                        rhs = qT[32 * j: 32 * j + 32,
                                 g * L + lh * 512: g * L + (lh + 1) * 512].bitcast(f32r)
                        nc.tensor.matmul(score[:, jj * 512:(jj + 1) * 512],
                                         lhsT, rhs, start=True, stop=True,
                                         tile_position=(32 * j, 0))
                    return score

                def exp_unit(score):
                    ptc = ptp.tile([128, 1024], bf16, tag="pt")
                    exp_ctr[0] += 1
                    if DVE_EXP_EVERY and exp_ctr[0] % DVE_EXP_EVERY == 0:
                        nc.vector.tensor_scalar(
                            out=ptc[:].bitcast(i16), in0=score[:],
                            scalar1=SCH_A, scalar2=SCH_B,
                            op0=ALU.mult, op1=ALU.add)
                    else:
                        nc.scalar.activation(ptc[:], score[:], AF.Exp, scale=ATT_SCALE)
                    return ptc

                def av_unit(av, den, ptc, g, tc_i, jp):
                    for jj in range(2):
                        j = 2 * jp + jj
                        rhs_pt = ptc[:, jj * 512:(jj + 1) * 512]
                        lhsT_v = v_sb[:, tc_i * C + g * 128 + j * 32:
                                      tc_i * C + g * 128 + (j + 1) * 32]
                        nc.tensor.matmul(av[32 * j: 32 * j + 32, :], lhsT_v, rhs_pt,
                                         start=False, stop=False,
                                         tile_position=(0, 32 * j),
                                         skip_group_check=True)
                    for jj in range(2):
                        j = 2 * jp + jj
                        rhs_pt = ptc[:, jj * 512:(jj + 1) * 512]
                        nc.tensor.matmul(den[32 * j: 32 * j + 32, :], ones32[:], rhs_pt,
                                         start=False, stop=False,
                                         tile_position=(0, 32 * j),
                                         skip_group_check=True)

                def new_avden():
                    av = psp.tile([128, 512], f32, tag="ps")
                    den = psp.tile([128, 512], f32, tag="ps")
                    nc.vector.memset(av[:], 0.0)
                    nc.vector.memset(den[:], 0.0)
                    return av, den

                def norm_block(av, den, g, lh):
                    rep = repp.tile([128, 512], f32, tag="rep")
                    nc.vector.reciprocal_approx_fast(out=rep[:], in_=den[:])
                    nc.vector.tensor_mul(
                        attn_oT[:, g * L + lh * 512: g * L + (lh + 1) * 512].bitcast(f32r),
                        av[:], rep[:])

                units = [(t, jp) for t in range(8) for jp in range(2)]

                # --- block (g=0, lh=0): exps buffered, attn@V deferred ---
                # conv_v quarters (co,half) = (0,1),(1,0),(1,1) slot between
                # units so the PE keeps pace with the ScalarE exp stream.
                fillers = {
                    4: lambda: conv_quarter(pad_y, wv_sb, vraw, 4, 0, 1),
                    9: lambda: conv_quarter(pad_y, wv_sb, vraw, 4, 1, 0),
                    13: lambda: conv_quarter(pad_y, wv_sb, vraw, 4, 1, 1),
                }
                backlog = []
                for u, (t, jp) in enumerate(units):
                    score = sc_unit(0, 0, t, jp)
                    backlog.append((exp_unit(score), t, jp))
                    if u in fillers:
                        fillers[u]()
                emit_cc2_bn_v()
                # keep the exp stream fed while v catches up
                lead = [(exp_unit(sc_unit(0, 1, t, jp)), t, jp)
                        for (t, jp) in units[:2]]
                emit_vproj()
                av0, den0 = new_avden()
                for ptc, t, jp in backlog:
                    av_unit(av0, den0, ptc, 0, t, jp)
                norm_block(av0, den0, 0, 0)

                # --- remaining blocks: lag-3 pipeline ---
                def run_block(g, lh, pending):
                    av, den = new_avden()
                    for (t, jp) in units[len(pending):]:
                        score = sc_unit(g, lh, t, jp)
                        pending.append((exp_unit(score), t, jp))
                        if len(pending) > 3:
                            ptc, pt_, pjp = pending.pop(0)
                            av_unit(av, den, ptc, g, pt_, pjp)
                    while pending:
                        ptc, pt_, pjp = pending.pop(0)
                        av_unit(av, den, ptc, g, pt_, pjp)
                    norm_block(av, den, g, lh)

                run_block(0, 1, lead)
                run_block(1, 0, [])
                run_block(1, 1, [])

                # ---------- output projection (transposed) + bias ----------
                out_sb = singles.tile([128, 2 * L], f32)
                for lh in range(2):
                    for co in range(2):
                        ps = psp.tile([128, 512], f32, tag="ps")
                        for ci in range(2):
                            lhsT = po_sb[:, ci * C + co * 128: ci * C + (co + 1) * 128]
                            rhs = attn_oT[:, ci * L + lh * 512: ci * L + (lh + 1) * 512].bitcast(f32r)
                            nc.tensor.matmul(ps[:], lhsT, rhs, start=(ci == 0), stop=(ci == 1))
                        nc.vector.tensor_scalar(
                            out=out_sb[:, co * L + lh * 512: co * L + (lh + 1) * 512],
                            in0=ps[:], scalar1=bot[:, co: co + 1], scalar2=None,
                            op0=ALU.add)

                outr = out.rearrange("(c p) l -> p c l", p=128)
                osr = out_sb[:].rearrange("p (c l) -> p c l", l=L)
                for lh in range(2):
                    nc.sync.dma_start(out=outr[:, :, lh * 512:(lh + 1) * 512],
                                      in_=osr[:, :, lh * 512:(lh + 1) * 512])

    nc.compile()
    return nc


def _prep_weights(conv_q_w, conv_k_w, conv_v_w, Wq, Wk, Wv, Wo,
                  bn_q_g, bn_q_b, bn_k_g, bn_k_b, bn_v_g, bn_v_b, bo):
    if CONV_FP8:
        from concourse import mybir
        np_fp8 = mybir.dt.np(mybir.dt.float8e4)

    def conv_tiles(w):
        # [co, ci, ky, kx] -> [9, 2(ci), 2(co), 128, 128]
        t = np.ascontiguousarray(np.transpose(np.asarray(w, np.float32), (2, 3, 1, 0)))
        t = t.reshape(3, 3, 2, 128, 2, 128).transpose(0, 1, 2, 4, 3, 5)
        t = np.ascontiguousarray(t.reshape(9, 2, 2, 128, 128))
        if CONV_FP8:
            t = (t * W8_SCALE).astype(np_fp8)
        return t

    def proj_tiles(w):
        return np.ascontiguousarray(
            np.asarray(w, np.float32).T.reshape(2, 128, C))

    gbp = np.zeros((128, 12), np.float32)
    for i, (g, b) in enumerate(((bn_q_g, bn_q_b), (bn_k_g, bn_k_b), (bn_v_g, bn_v_b))):
        g = np.asarray(g, np.float32).reshape(2, 128)
        b = np.asarray(b, np.float32).reshape(2, 128)
        for ch in range(2):
            gbp[:, 2 * i + ch] = g[ch]
            gbp[:, 6 + 2 * i + ch] = b[ch]
    bop = np.ascontiguousarray(np.asarray(bo, np.float32).reshape(2, 128).T)
    return {
        "wcq": conv_tiles(conv_q_w), "wck": conv_tiles(conv_k_w),
        "wcv": conv_tiles(conv_v_w),
        "pq": proj_tiles(Wq), "pk": proj_tiles(Wk), "pv": proj_tiles(Wv),
        "po": proj_tiles(Wo),
        "gb": gbp, "bo": bop,
    }


def _get_nc(repeat=1):
    key = ("nc", repeat, VARIANT, DEBUG, RSTD_MODE, DVE_EXP_EVERY)
    if key not in _CACHE:
        _CACHE[key] = _build_nc(repeat)
    return _CACHE[key]


def run_spmd(in_maps, repeat=1, **kw):
    from concourse.bass_utils import run_bass_kernel_spmd
    return run_bass_kernel_spmd(_get_nc(repeat), in_maps, list(range(8)), **kw)


def _get_executor(repeat=1):
    """Build the sharded jitted callable once (mirrors
    bass2jax.run_bass_via_pjrt's multi-core path) so repeated calls skip
    retracing/compilation."""
    key = ("exec", repeat, VARIANT, RSTD_MODE, DVE_EXP_EVERY)
    if key in _CACHE:
        return _CACHE[key]
    import jax
    import numpy as _np
    from jax.sharding import Mesh, PartitionSpec
    from jax.experimental.shard_map import shard_map
    from concourse import bass2jax, mybir

    nc = _get_nc(repeat)
    bass2jax.install_neuronx_cc_hook()
    partition_name = nc.partition_id_tensor.name if nc.partition_id_tensor else None

    in_names, out_names, out_avals, zero_outs = [], [], [], []
    for alloc in nc.m.functions[0].allocations:
        if not isinstance(alloc, mybir.MemoryLocationSet):
            continue
        name = alloc.memorylocations[0].name
        if alloc.kind == "ExternalInput":
            if name != partition_name:
                in_names.append(name)
        elif alloc.kind == "ExternalOutput":
            dt_np = mybir.dt.np(alloc.dtype)
            shape = tuple(alloc.tensor_shape)
            out_avals.append(jax.core.ShapedArray(shape, dt_np))
            out_names.append(name)
            zero_outs.append(_np.zeros(shape, dt_np))

    n_params = len(in_names)
    n_outs = len(out_names)
    all_in_names = list(in_names) + list(out_names)
    if partition_name is not None:
        all_in_names.append(partition_name)
    donate = tuple(range(n_params, n_params + n_outs))

    def _body(*args):
        operands = list(args)
        if partition_name is not None:
            operands.append(bass2jax.partition_id_tensor())
        outs = bass2jax._bass_exec_p.bind(
            *operands,
            out_avals=tuple(out_avals),
            in_names=tuple(all_in_names),
            out_names=tuple(out_names),
            lowering_input_output_aliases=(),
            sim_require_finite=True,
            sim_require_nnan=True,
            nc=nc,
        )
        return tuple(outs)

    devices = jax.devices()[:B]
    mesh = Mesh(np.asarray(devices), ("core",))
    in_specs = (PartitionSpec("core"),) * (n_params + n_outs)
    out_specs = (PartitionSpec("core"),) * n_outs
    sharded = jax.jit(
        shard_map(_body, mesh=mesh, in_specs=in_specs, out_specs=out_specs,
                  check_rep=False),
        donate_argnums=donate, keep_unused=True,
    )
    _CACHE[("mesh", repeat, VARIANT)] = mesh
    _CACHE[("jit", repeat, VARIANT)] = sharded

    def run(in_maps):
        concat_in = [
            np.concatenate([np.asarray(in_maps[c][k]) for c in range(B)], axis=0)
            for k in in_names
        ]
        concat_zeros = [np.zeros((B * z.shape[0], *z.shape[1:]), z.dtype)
                        for z in zero_outs]
        out_arrs = sharded(*concat_in, *concat_zeros)
        return out_arrs, out_names, out_avals

    _CACHE[key] = run
    return run


def run_fast(in_maps, repeat=1):
    """Execute via the cached jitted callable; returns per-core dict list."""
    run = _get_executor(repeat)
    out_arrs, out_names, out_avals = run(in_maps)
    return [
        {name: np.asarray(out_arrs[i]).reshape(B, *out_avals[i].shape)[c]
         for i, name in enumerate(out_names)}
        for c in range(B)
    ]


def bench_wall(in_maps, repeat, n_iter):
    """Dispatch n_iter executions of the repeat-R NEFF with device-resident
    inputs and pre-staged donated zero buffers; return total wall seconds."""
    import time as _time
    import jax
    from jax.sharding import NamedSharding, PartitionSpec

    _get_executor(repeat)  # ensure built
    nc = _get_nc(repeat)
    from concourse import mybir
    partition_name = nc.partition_id_tensor.name if nc.partition_id_tensor else None
    in_names, out_shapes = [], []
    for alloc in nc.m.functions[0].allocations:
        if not isinstance(alloc, mybir.MemoryLocationSet):
            continue
        name = alloc.memorylocations[0].name
        if alloc.kind == "ExternalInput" and name != partition_name:
            in_names.append(name)
        elif alloc.kind == "ExternalOutput":
            out_shapes.append((tuple(alloc.tensor_shape), mybir.dt.np(alloc.dtype)))

    key = ("bench_in", repeat, VARIANT)
    if key not in _CACHE:
        mesh = _CACHE[("mesh", repeat, VARIANT)]
        sh = NamedSharding(mesh, PartitionSpec("core"))
        dev_in = [
            jax.device_put(
                np.concatenate([np.asarray(in_maps[c][k]) for c in range(B)], 0), sh)
            for k in in_names
        ]
        _CACHE[key] = (dev_in, sh)
    dev_in, sh = _CACHE[key]

    sharded = _CACHE[("jit", repeat, VARIANT)]
    zero_sets = []
    for _ in range(n_iter):
        zs = [jax.device_put(np.zeros((B * s[0], *s[1:]), dt), sh)
              for (s, dt) in out_shapes]
        zero_sets.append(zs)
    for zs in zero_sets:
        for z in zs:
            z.block_until_ready()

    outs = []
    t0 = _time.perf_counter()
    for it in range(n_iter):
        outs.append(sharded(*dev_in, *zero_sets[it]))
    for o in outs[-1]:
        o.block_until_ready()
    t1 = _time.perf_counter()
    return t1 - t0


def bench_alternating(in_maps, r1, r2, n_pairs):
    """Alternate single dispatches of the repeat-r1 and repeat-r2 NEFFs,
    blocking after each; per-iter ns = (median(w2) - median(w1)) /
    (r2 - r1). Alternation cancels slow host/RPC drift; medians kill
    spikes."""
    import time as _time
    import jax
    from jax.sharding import NamedSharding, PartitionSpec

    from concourse import mybir

    def setup(repeat):
        _get_executor(repeat)
        nc = _get_nc(repeat)
        pn = nc.partition_id_tensor.name if nc.partition_id_tensor else None
        in_names, out_shapes = [], []
        for alloc in nc.m.functions[0].allocations:
            if not isinstance(alloc, mybir.MemoryLocationSet):
                continue
            name = alloc.memorylocations[0].name
            if alloc.kind == "ExternalInput" and name != pn:
                in_names.append(name)
            elif alloc.kind == "ExternalOutput":
                out_shapes.append(
                    (tuple(alloc.tensor_shape), mybir.dt.np(alloc.dtype)))
        mesh = _CACHE[("mesh", repeat, VARIANT)]
        sh = NamedSharding(mesh, PartitionSpec("core"))
        dev_in = [
            jax.device_put(
                np.concatenate([np.asarray(in_maps[c][k]) for c in range(B)], 0),
                sh)
            for k in in_names
        ]
        sharded = _CACHE[("jit", repeat, VARIANT)]
        return sharded, dev_in, out_shapes, sh

    s1, din1, osh1, sh1 = setup(r1)
    s2, din2, osh2, sh2 = setup(r2)

    def zeros_for(osh, sh):
        zs = [jax.device_put(np.zeros((B * s[0], *s[1:]), dt), sh)
              for (s, dt) in osh]
        for z in zs:
            z.block_until_ready()
        return zs

    BATCH = 10

    def batch(sharded, dev_in, zsets):
        t0 = _time.perf_counter()
        outs = []
        for zs in zsets:
            outs.append(sharded(*dev_in, *zs))
        for o in outs[-1]:
            o.block_until_ready()
        return _time.perf_counter() - t0

    # warm both
    batch(s1, din1, [zeros_for(osh1, sh1) for _ in range(2)])
    batch(s2, din2, [zeros_for(osh2, sh2) for _ in range(2)])
    w1, w2 = [], []
    for _ in range(n_pairs):
        z1 = [zeros_for(osh1, sh1) for _ in range(BATCH)]
        z2 = [zeros_for(osh2, sh2) for _ in range(BATCH)]
        w1.append(batch(s1, din1, z1))
        w2.append(batch(s2, din2, z2))
    w1 = np.asarray(w1)
    w2 = np.asarray(w2)
    scale = 1e9 / ((r2 - r1) * BATCH)
    med = (np.median(w2) - np.median(w1)) * scale
    lo = (np.percentile(w2, 25) - np.percentile(w1, 75)) * scale
    hi = (np.percentile(w2, 75) - np.percentile(w1, 25)) * scale
    return med, lo, hi


def bench_min(in_maps, repeats=(1, 9), n=100):
    """Individually time n dispatches of each repeat-R NEFF, interleaved.
    Returns {R: sorted walls}. min(w_R2)-min(w_R1) cancels the RPC floor."""
    import time as _time
    import jax
    from jax.sharding import NamedSharding, PartitionSpec

    from concourse import mybir

    setups = {}
    for repeat in repeats:
        _get_executor(repeat)
        nc = _get_nc(repeat)
        pn = nc.partition_id_tensor.name if nc.partition_id_tensor else None
        in_names, out_shapes = [], []
        for alloc in nc.m.functions[0].allocations:
            if not isinstance(alloc, mybir.MemoryLocationSet):
                continue
            name = alloc.memorylocations[0].name
            if alloc.kind == "ExternalInput" and name != pn:
                in_names.append(name)
            elif alloc.kind == "ExternalOutput":
                out_shapes.append(
                    (tuple(alloc.tensor_shape), mybir.dt.np(alloc.dtype)))
        mesh = _CACHE[("mesh", repeat, VARIANT)]
        sh = NamedSharding(mesh, PartitionSpec("core"))
        dev_in = [
            jax.device_put(
                np.concatenate([np.asarray(in_maps[c][k]) for c in range(B)], 0),
                sh)
            for k in in_names
        ]
        sharded = _CACHE[("jit", repeat, VARIANT)]
        setups[repeat] = (sharded, dev_in, out_shapes, sh)

    def zeros_for(osh, sh):
        zs = [jax.device_put(np.zeros((B * s[0], *s[1:]), dt), sh)
              for (s, dt) in osh]
        for z in zs:
            z.block_until_ready()
        return zs

    walls = {r: [] for r in repeats}
    # warm
    for r in repeats:
        sharded, dev_in, osh, sh = setups[r]
        outs = sharded(*dev_in, *zeros_for(osh, sh))
        for o in outs:
            o.block_until_ready()
    for _ in range(n):
        for r in repeats:
            sharded, dev_in, osh, sh = setups[r]
            zs = zeros_for(osh, sh)
            t0 = _time.perf_counter()
            outs = sharded(*dev_in, *zs)
            for o in outs:
                o.block_until_ready()
            walls[r].append(_time.perf_counter() - t0)
    return {r: np.sort(np.asarray(w)) for r, w in walls.items()}


def make_in_maps(x, y, h, w, conv_q_w, bn_q_g, bn_q_b,
                 conv_k_w, bn_k_g, bn_k_b, conv_v_w, bn_v_g, bn_v_b,
                 Wq, Wk, Wv, Wo, bo):
    assert int(h) == IMG and int(w) == IMG
    x = np.asarray(x, np.float32)
    y = np.asarray(y, np.float32)
    wmap = _prep_weights(conv_q_w, conv_k_w, conv_v_w, Wq, Wk, Wv, Wo,
                         bn_q_g, bn_q_b, bn_k_g, bn_k_b, bn_v_g, bn_v_b, bo)
    def pad_t(a):
        # [B, L, C] -> [B, C, 34*34] with zero border baked in
        at = np.transpose(a, (0, 2, 1)).reshape(B, C, IMG, IMG)
        ap = np.zeros((B, C, PAD, PAD), np.float32)
        ap[:, :, 1:33, 1:33] = at
        return ap.reshape(B, C, PAD * PAD)

    xT = pad_t(x)
    yT = pad_t(y)
    return [dict(wmap, xt=xT[b], yt=yT[b]) for b in range(B)]


def kernel(**inputs):
    in_maps = make_in_maps(**inputs)
    res = run_fast(in_maps)
    outs = [res[b]["out"] for b in range(B)]  # each [C, L]
    return np.ascontiguousarray(
        np.stack(outs, axis=0).transpose(0, 2, 1)).astype(np.float32)


# revision 51
# speedup vs baseline: 1.1316x; 1.1316x over previous
"""Distributed Trainium2 (Bass/Tile) kernel for nn_Attention_2D.

Pipeline (per batch element): 3x3 conv + BatchNorm (batch stats!) for
Q (from x), K, V (from y) -> linear projections -> multi-head attention
(scale = C**-0.5) -> output projection.

Sharding: data-parallel over batch B=8 across the 8 NeuronCores (one
image per core). The only cross-core dependency is the BatchNorm
mean/var over the whole batch -> tiny [128,{8,4}] AllReduces.

Schedule (PE matmuls execute serially on this HW path - tile_position
packing does NOT overlap them - so the design minimizes total streamed
matmul rows and keeps the ScalarE exp stream, 64 x ~1.15us, fed):
  - scores stay in PSUM ([128,1024] tiles) and ScalarE exps them
    PSUM->SBUF bf16 directly (no staging copies).
  - Q/K convs run in fp8e4m3 with DoubleRow (two input-channel chunks
    packed into the contraction -> half the PE cycles).  Host scales
    the weights by W8_SCALE to clear fp8 subnormals; BatchNorm is
    per-channel scale-invariant so nothing downstream changes.  The V
    conv stays f32r: V's error passes straight to the output (the
    softmax path averages errors away, measured 3e-3; fp8 V measures
    3e-2).
  - the attention denominator is FREE: V is augmented with 32 ones
    columns, so each attn@V matmul yields [64,512] = 32 rows of P@V
    plus 32 rows of replicated sum(P).  Two heads share a PSUM bank
    (base partitions 0/64).  A DRAM bounce (idle DMA rings) repacks
    per-head banks into attention layout and broadcasts nothing -- the
    denominator rows are already replicated.
  - emission order: conv_k, conv_q, CC1 covered by a quarter of
    conv_v, BN (DVE Newton rsqrt - no act-table thrash), q/k
    projections, then attention; the rest of conv_v + CC2 + v
    projection interleave into the first attention block, whose attn@V
    is deferred (exp tiles buffered) until V lands.
"""

import os

import numpy as np

B, L, C = 8, 1024, 256
H = 8
D = 32  # head dim
IMG = 32  # h = w = 32
PAD = 34  # padded image side
EPS = 1e-5
ATT_SCALE = float(C) ** -0.5  # 1/16

# Schraudolph exp in bf16-bits: exp(s*x) ~= bitcast_bf16(i16(x*SCH_A + SCH_B))
SCH_A = (128.0 / float(np.log(2.0))) * ATT_SCALE
SCH_B = 128.0 * (127.0 - 0.0450466)

_CACHE = {}
DEBUG = False
VARIANT = "full"  # "full" | "noattn" | "convonly" (phase timing builds)
SIM_NO_CC = False  # replace AllReduce with local DMA copy (TimelineSim only)
RSTD_MODE = "newton"  # "newton" (DVE) | "lnexp" (ScalarE tables)
# 0 = off; n>0: every nth exp tile on DVE (Schraudolph)
DVE_EXP_EVERY = int(os.environ.get("DVE_EXP_EVERY", "0"))
# fp8e4m3 DoubleRow for the Q/K convs (see module docstring)
CONV_FP8 = int(os.environ.get("CONV_FP8", "1"))
W8_SCALE = 64.0
PADW8 = 40  # fp8 padded image row width (ci stride 34*40 B, 16-aligned)


def _build_nc(repeat=1):
    import concourse.bacc as bacc
    import concourse.tile as tile
    from concourse import mybir

    f32 = mybir.dt.float32
    f32r = mybir.dt.float32r
    bf16 = mybir.dt.bfloat16
    i16 = mybir.dt.int16
    i32 = mybir.dt.int32
    fp8 = mybir.dt.float8e4
    AF = mybir.ActivationFunctionType
    ALU = mybir.AluOpType

    nc = bacc.Bacc(None, target_bir_lowering=False)
    nc.num_devices = 8

    # ---- DRAM parameters (host-prepped layouts) ----
    if CONV_FP8:
        xt = nc.declare_dram_parameter("xt", [C, 34 * PADW8], fp8, isOutput=False)
        yt8 = nc.declare_dram_parameter("yt8", [C, 34 * PADW8], fp8, isOutput=False)
        wcq = nc.declare_dram_parameter("wcq", [9, 2, 2, 128, 128], fp8, isOutput=False)
        wck = nc.declare_dram_parameter("wck", [9, 2, 2, 128, 128], fp8, isOutput=False)
    else:
        xt = nc.declare_dram_parameter("xt", [C, PAD * PAD], f32r, isOutput=False)
        yt8 = None
        wcq = nc.declare_dram_parameter("wcq", [9, 2, 2, 128, 128], f32r, isOutput=False)
        wck = nc.declare_dram_parameter("wck", [9, 2, 2, 128, 128], f32r, isOutput=False)
    ytf = nc.declare_dram_parameter("ytf", [C, PAD * PAD], f32r, isOutput=False)
    wcv = nc.declare_dram_parameter("wcv", [9, 2, 2, 128, 128], f32r, isOutput=False)
    # projection weights W.T tiled: [2(ci), 128, 256(co)]
    pq = nc.declare_dram_parameter("pq", [2, 128, C], f32r, isOutput=False)
    pk = nc.declare_dram_parameter("pk", [2, 128, C], f32r, isOutput=False)
    pv = nc.declare_dram_parameter("pv", [2, 128, C], f32r, isOutput=False)
    po = nc.declare_dram_parameter("po", [2, 128, C], f32r, isOutput=False)
    # gamma/beta pack [128, 12]: cols 0-5 gamma, 6-11 beta
    gb = nc.declare_dram_parameter("gb", [128, 12], f32, isOutput=False)
    bo = nc.declare_dram_parameter("bo", [128, 2], f32, isOutput=False)
    out = nc.declare_dram_parameter("out", [C, L], f32, isOutput=True)
    dbg = {}
    if os.environ.get("DEBUG_OUT"):
        for nm, shape, dt_ in (("dqT", [128, 2 * L], f32), ("dkT", [128, 2 * L], f32),
                               ("dvsb", [128, 8 * C], mybir.dt.bfloat16),
                               ("daoT", [128, 2 * L], f32)):
            dbg[nm] = nc.declare_dram_parameter(nm, shape, dt_, isOutput=True)

    with tile.TileContext(nc) as tc:
        with tc.tile_pool(name="singles", bufs=1) as singles, \
             tc.tile_pool(name="stats", bufs=1) as statsp, \
             tc.tile_pool(name="bnst", bufs=4) as bnstp, \
             tc.tile_pool(name="rep", bufs=2) as repp, \
             tc.tile_pool(name="nrm", bufs=4) as nrmp, \
             tc.tile_pool(name="pt", bufs=(24 if CONV_FP8 else 20)) as ptp, \
             tc.tile_pool(name="ps", bufs=4, space="PSUM") as psp, \
             tc.tile_pool(name="score_ps", bufs=2, space="PSUM") as scorep, \
             tc.tile_pool(name="dram", bufs=4, space="DRAM") as dramp:

            for _rep in range(repeat):
                # ---------- constants / small tiles ----------
                epst = singles.tile([128, 1], f32)
                nc.vector.memset(epst[:], EPS)
                magict = singles.tile([128, 6], f32)
                # f32 whose bits are 0x5f375a86 (Newton-rsqrt magic)
                nc.vector.memset(magict[:], 1.3212019791402893e19)
                # augmented V for attn@V: [t, tc, g*4+j, 64]; cols 0-31 get
                # head (g,j)'s V via DMA scatter, cols 32-63 stay 1.0 so the
                # same matmul emits 32 replicated denominator rows.
                v_aug = singles.tile([128, 8, 8, 64], bf16)
                nc.vector.memset(v_aug[:], 1.0)
                gbt = singles.tile([128, 12], f32)
                nc.sync.dma_start(out=gbt[:], in_=gb[:])
                bot = singles.tile([128, 2], f32)
                nc.sync.dma_start(out=bot[:], in_=bo[:])

                # ---------- padded images + weights ----------
                cw_dt = fp8 if CONV_FP8 else f32r
                if CONV_FP8:
                    pad_x = singles.tile([128, 2, 34, PADW8], fp8)
                    pad_y8 = singles.tile([128, 2, 34, PADW8], fp8)
                    xtr = xt.rearrange("(c p) (r w) -> p c r w", p=128, w=PADW8)
                    ytr8 = yt8.rearrange("(c p) (r w) -> p c r w", p=128, w=PADW8)
                else:
                    pad_x = singles.tile([128, 2, PAD, PAD], f32r)
                    xtr = xt.rearrange("(c p) m -> p c m", p=128)
                    pad_y8 = None
                pad_yf = singles.tile([128, 2, PAD, PAD], f32r)
                ytrf = ytf.rearrange("(c p) m -> p c m", p=128)
                wq_sb = singles.tile([128, 36 * 128], cw_dt)
                wk_sb = singles.tile([128, 36 * 128], cw_dt)
                wv_sb = singles.tile([128, 36 * 128], f32r)
                pq_sb = singles.tile([128, 2 * C], f32r)
                pk_sb = singles.tile([128, 2 * C], f32r)
                pv_sb = singles.tile([128, 2 * C], f32r)
                po_sb = singles.tile([128, 2 * C], f32r)

                wk4 = wk_sb[:].rearrange("p (a t f) -> p a t f", a=3, f=128)
                wq4 = wq_sb[:].rearrange("p (a t f) -> p a t f", a=3, f=128)
                wv4 = wv_sb[:].rearrange("p (a t f) -> p a t f", a=3, f=128)
                wckr = wck.rearrange("(a g) b c p f -> p a (g b c) f", a=3)
                wcqr = wcq.rearrange("(a g) b c p f -> p a (g b c) f", a=3)
                wcvr = wcv.rearrange("(a g) b c p f -> p a (g b c) f", a=3)
                # two HWDGE rings, FIFO each; emit in consumption order.
                if CONV_FP8:
                    for ci in range(2):
                        nc.sync.dma_start(out=pad_y8[:, ci], in_=ytr8[:, ci])
                else:
                    for ci in range(2):
                        nc.sync.dma_start(out=pad_yf[:, ci], in_=ytrf[:, ci])
                for a in range(3):
                    nc.scalar.dma_start(out=wk4[:, a], in_=wckr[:, a])
                for a in range(3):
                    nc.sync.dma_start(out=wq4[:, a], in_=wcqr[:, a])
                for ci in range(2):
                    nc.scalar.dma_start(out=pad_x[:, ci], in_=xtr[:, ci])
                for eng, psb, pdr in ((nc.sync, pk_sb, pk), (nc.scalar, pq_sb, pq)):
                    eng.dma_start(
                        out=psb[:].rearrange("p (t f) -> p t f", f=C),
                        in_=pdr.rearrange("t p f -> p t f"),
                    )
                if CONV_FP8:
                    for ci in range(2):
                        nc.scalar.dma_start(out=pad_yf[:, ci], in_=ytrf[:, ci])
                for a in range(3):
                    nc.scalar.dma_start(out=wv4[:, a], in_=wcvr[:, a])
                nc.sync.dma_start(
                    out=pv_sb[:].rearrange("p (t f) -> p t f", f=C),
                    in_=pv.rearrange("t p f -> p t f"),
                )
                nc.scalar.dma_start(
                    out=po_sb[:].rearrange("p (t f) -> p t f", f=C),
                    in_=po.rearrange("t p f -> p t f"),
                )

                # ---------- conv machinery ----------
                kraw = singles.tile([128, 2 * L], f32)
                vraw = singles.tile([128, 2 * L], f32)
                qraw = singles.tile([128, 2 * L], f32)
                st = statsp.tile([128, 12], f32)  # local (mean, m2) pairs

                def conv_quarter(pad_t, w_sb, raw, stat_base, co, half, use8):
                    ps = psp.tile([128, 512], f32, tag="ps")
                    if use8:
                        w8v = w_sb[:].rearrange(
                            "p (kp ci co f) -> p kp ci co f", kp=9, ci=2, co=2)
                        for kp in range(9):
                            ky, kx = kp // 3, kp % 3
                            lhsT = w8v[:, kp, :, co, :]
                            rhs = pad_t[:, :, ky + half * 16: ky + half * 16 + 16,
                                        kx: kx + 32]
                            nc.tensor.matmul(
                                ps[:], lhsT, rhs, start=(kp == 0), stop=(kp == 8),
                                perf_mode=mybir.MatmulPerfMode.DoubleRow)
                    else:
                        idx = 0
                        for kp in range(9):
                            ky, kx = kp // 3, kp % 3
                            for ci in range(2):
                                blk = (kp * 2 + ci) * 2 + co
                                lhsT = w_sb[:, blk * 128:(blk + 1) * 128]
                                rhs = pad_t[:, ci, ky + half * 16: ky + half * 16 + 16,
                                            kx: kx + 32]
                                nc.tensor.matmul(ps[:], lhsT, rhs,
                                                 start=(idx == 0), stop=(idx == 17))
                                idx += 1
                    nc.vector.tensor_copy(
                        out=raw[:, co * L + half * 512: co * L + (half + 1) * 512].bitcast(f32r),
                        in_=ps[:])
                    if half == 1:
                        k = stat_base + co
                        st6 = bnstp.tile([128, 2, 6], f32, tag="st6")
                        nc.vector.bn_stats(st6[:, 0, :], raw[:, co * L: co * L + 512])
                        nc.vector.bn_stats(st6[:, 1, :], raw[:, co * L + 512: co * L + 1024])
                        nc.vector.bn_aggr(st[:, 2 * k: 2 * k + 2], st6[:])
                        # m2 = mean^2 + var (in place on the var column)
                        nc.vector.scalar_tensor_tensor(
                            out=st[:, 2 * k + 1: 2 * k + 2],
                            in0=st[:, 2 * k: 2 * k + 1],
                            scalar=st[:, 2 * k: 2 * k + 1],
                            in1=st[:, 2 * k + 1: 2 * k + 2],
                            op0=ALU.mult, op1=ALU.add,
                        )

                def conv_full(pad_t, w_sb, raw, stat_base, use8):
                    for co in range(2):
                        for half in range(2):
                            conv_quarter(pad_t, w_sb, raw, stat_base, co, half, use8)

                gstats = statsp.tile([128, 12], f32)

                def emit_cc(c0, c1, name):
                    # stat moves ride the idle SWDGE (gpsimd) queue - the
                    # HWDGE rings are busy with multi-MB input loads early on
                    cc_in = dramp.tile([128, c1 - c0], f32, name=f"cci_{name}")
                    cc_out = dramp.tile([128, c1 - c0], f32, name=f"cco_{name}")
                    nc.sync.dma_start(out=cc_in[:], in_=st[:, c0:c1])
                    if SIM_NO_CC:
                        nc.gpsimd.dma_start(out=cc_out[:], in_=cc_in[:])
                    else:
                        nc.gpsimd.collective_compute(
                            "AllReduce", ALU.add,
                            replica_groups=[list(range(8))],
                            ins=[cc_in[:].opt()], outs=[cc_out[:].opt()],
                        )
                    nc.sync.dma_start(out=gstats[:, c0:c1], in_=cc_out[:])

                pad_yk = pad_y8 if CONV_FP8 else pad_yf
                conv_full(pad_yk, wk_sb, kraw, 2, CONV_FP8)
                emit_cc(4, 8, "k")   # K stats; latency hides under conv_q
                conv_full(pad_x, wq_sb, qraw, 0, CONV_FP8)
                emit_cc(0, 4, "q")   # Q stats; latency hides under conv_v q1

                # first quarter of conv_v covers the CC_q latency
                if VARIANT == "full":
                    conv_quarter(pad_yf, wv_sb, vraw, 4, 0, 0, False)
                else:
                    conv_full(pad_yf, wv_sb, vraw, 4, False)

                # ---------- global scale/shift ----------
                var_t = statsp.tile([128, 6], f32)
                nwt_h = statsp.tile([128, 6], f32)
                nwt_y = statsp.tile([128, 6], f32)
                nwt_t = statsp.tile([128, 6], f32)
                scale_t = statsp.tile([128, 6], f32)
                shift_t = statsp.tile([128, 6], f32)

                def bn_post(k0, nk):
                    seg = gstats[:, 2 * k0: 2 * (k0 + nk)]
                    nc.vector.tensor_scalar_mul(seg, seg, 1.0 / 8.0)
                    g2 = seg.rearrange("p (k two) -> p k two", two=2)
                    gmean = g2[:, :, 0]
                    gm2 = g2[:, :, 1]
                    vt = var_t[:, k0: k0 + nk]
                    nc.vector.tensor_mul(vt, gmean, gmean)
                    nc.vector.tensor_sub(vt, gm2, vt)
                    if RSTD_MODE == "newton":
                        # rstd = rsqrt(var+eps): bit-trick seed + 2 Newton steps
                        nc.vector.tensor_scalar_add(vt, vt, EPS)
                        hv = nwt_h[:, k0: k0 + nk]
                        nc.vector.tensor_scalar_mul(hv, vt, 0.5)
                        yv = nwt_y[:, k0: k0 + nk]
                        nc.vector.tensor_scalar(
                            out=yv.bitcast(i32), in0=vt.bitcast(i32),
                            scalar1=1, scalar2=None, op0=ALU.logical_shift_right)
                        nc.vector.tensor_sub(
                            yv.bitcast(i32), magict[:, k0: k0 + nk].bitcast(i32),
                            yv.bitcast(i32))
                        tv = nwt_t[:, k0: k0 + nk]
                        for _ in range(2):
                            nc.vector.tensor_mul(tv, yv, yv)
                            nc.vector.tensor_mul(tv, tv, hv)
                            nc.vector.tensor_scalar(
                                out=tv, in0=tv, scalar1=-1.0, scalar2=1.5,
                                op0=ALU.mult, op1=ALU.add)
                            nc.vector.tensor_mul(yv, yv, tv)
                        vt = yv
                    else:
                        nc.scalar.activation(vt, vt, AF.Ln, bias=epst[:, 0:1], scale=1.0)
                        nc.scalar.activation(vt, vt, AF.Exp, scale=-0.5)
                    sc = scale_t[:, k0: k0 + nk]
                    sh = shift_t[:, k0: k0 + nk]
                    nc.vector.tensor_mul(sc, vt, gbt[:, k0: k0 + nk])
                    nc.vector.tensor_mul(sh, gmean, sc)
                    nc.vector.tensor_sub(sh, gbt[:, 6 + k0: 6 + k0 + nk], sh)

                def bn_apply(raw, base):
                    for ch in range(2):
                        k = base + ch
                        nc.vector.tensor_scalar(
                            out=raw[:, ch * L:(ch + 1) * L].bitcast(f32r),
                            in0=raw[:, ch * L:(ch + 1) * L],
                            scalar1=scale_t[:, k: k + 1],
                            scalar2=shift_t[:, k: k + 1],
                            op0=ALU.mult, op1=ALU.add,
                        )

                bn_post(2, 2)   # k (CC_k result arrives first)
                bn_apply(kraw, 2)
                bn_post(0, 2)   # q
                bn_apply(qraw, 0)

                def emit_cc2_bn_v():
                    emit_cc(8, 12, "v")
                    bn_post(4, 2)
                    bn_apply(vraw, 4)

                if VARIANT == "convonly":
                    emit_cc2_bn_v()
                    nc.sync.dma_start(
                        out=out.rearrange("(c p) l -> p c l", p=128),
                        in_=kraw[:].rearrange("p (c l) -> p c l", l=L))
                    continue

                # ---------- q/k projections -> transposed [c, L] ----------
                qT = singles.tile([128, 2 * L], f32)
                kT = singles.tile([128, 2 * L], f32)

                def proj_T(src_t, wsb, dst, co):
                    for lh in range(2):
                        ps = psp.tile([128, 512], f32, tag="ps")
                        for ci in range(2):
                            lhsT = wsb[:, ci * C + co * 128: ci * C + (co + 1) * 128]
                            rhs = src_t[:, ci * L + lh * 512: ci * L + (lh + 1) * 512].bitcast(f32r)
                            nc.tensor.matmul(ps[:], lhsT, rhs,
                                             start=(ci == 0), stop=(ci == 1))
                        nc.vector.tensor_copy(
                            out=dst[:, co * L + lh * 512: co * L + (lh + 1) * 512].bitcast(f32r),
                            in_=ps[:])

                proj_T(kraw, pk_sb, kT, 0)
                proj_T(qraw, pq_sb, qT, 0)

                # ---------- v projection + masked scatter ----------
                v_sb = singles.tile([128, 8 * C], bf16)  # col = tc*256 + c

                def emit_vproj():
                    for lt in range(8):
                        ps = psp.tile([128, C], f32, tag="ps")
                        for ci in range(2):
                            lhsT = vraw[:, ci * L + lt * 128: ci * L + (lt + 1) * 128].bitcast(f32r)
                            rhs = pv_sb[:, ci * C:(ci + 1) * C]
                            nc.tensor.matmul(ps[:], lhsT, rhs, start=(ci == 0), stop=(ci == 1))
                        nc.vector.tensor_copy(out=v_sb[:, lt * C:(lt + 1) * C], in_=ps[:])
                    vsr = v_sb[:].rearrange("p (tc c) -> p tc c", c=C)
                    for g in range(2):
                        for j in range(4):
                            nc.scalar.dma_start(
                                out=v_aug[:, :, g * 4 + j, 0:32],
                                in_=vsr[:, :, g * 128 + 32 * j: g * 128 + 32 * (j + 1)],
                            )

                if VARIANT == "noattn":
                    emit_cc2_bn_v()
                    emit_vproj()
                    nc.sync.dma_start(
                        out=out.rearrange("(c p) l -> p c l", p=128),
                        in_=qT[:].rearrange("p (c l) -> p c l", l=L))
                    continue

                # ---------- attention ----------
                attn_oT = singles.tile([128, 2 * L], f32)  # col = g*1024 + l
                exp_ctr = [0]

                def sc_unit(g, lh, tc_i, jp):
                    score = scorep.tile([128, 1024], f32, tag="score")
                    for jj in range(2):
                        j = 2 * jp + jj
                        lhsT = kT[32 * j: 32 * j + 32,
                                  g * L + tc_i * 128: g * L + (tc_i + 1) * 128].bitcast(f32r)
                        rhs = qT[32 * j: 32 * j + 32,
                                 g * L + lh * 512: g * L + (lh + 1) * 512].bitcast(f32r)
                        nc.tensor.matmul(score[:, jj * 512:(jj + 1) * 512],
                                         lhsT, rhs, start=True, stop=True,
                                         tile_position=(32 * j, 0))
                    return score

                def exp_unit(score):
                    ptc = ptp.tile([128, 1024], bf16, tag="pt")
                    exp_ctr[0] += 1
                    if DVE_EXP_EVERY and exp_ctr[0] % DVE_EXP_EVERY == 0:
                        nc.vector.tensor_scalar(
                            out=ptc[:].bitcast(i16), in0=score[:],
                            scalar1=SCH_A, scalar2=SCH_B,
                            op0=ALU.mult, op1=ALU.add)
                    else:
                        nc.scalar.activation(ptc[:], score[:], AF.Exp, scale=ATT_SCALE)
                    return ptc

                # attn@V banks: one [64,512] bank per head, always written
                # at tile_position (0,0) - col-positioned PSUM accumulation
                # is corrupted by interleaved row-tiled score matmuls on this
                # HW.  rows 0-31 = P@V, rows 32-63 = replicated sum(P).
                def av_unit(banks, item, g):
                    hp, t, ptc = item
                    for jj in range(2):
                        j = 2 * hp + jj
                        rhs_pt = ptc[:, jj * 512:(jj + 1) * 512]
                        lhsT_v = v_aug[:, t, g * 4 + j, :]
                        nc.tensor.matmul(
                            banks[jj][:, :], lhsT_v, rhs_pt,
                            start=(t == 0), stop=(t == 7),
                            skip_group_check=True)

                def copy_half(banks, drsc, hp):
                    for jj in range(2):
                        tmp = nrmp.tile([64, 512], f32, tag="nrm", name="nrmt")
                        nc.vector.tensor_copy(out=tmp[:], in_=banks[jj][:])
                        nc.scalar.dma_start(out=drsc[2 * hp + jj], in_=tmp[:])

                def finish_norm(drsc, g, lh):
                    avs = nrmp.tile([128, 512], f32, tag="nrm", name="avs")
                    repw = repp.tile([128, 512], f32, tag="rep", name="repw")
                    rep = repp.tile([128, 512], f32, tag="rep", name="rep")
                    for j in range(4):
                        nc.sync.dma_start(out=avs[32 * j: 32 * j + 32, :],
                                          in_=drsc[j, 0:32, :])
                        nc.sync.dma_start(out=repw[32 * j: 32 * j + 32, :],
                                          in_=drsc[j, 32:64, :])
                    nc.vector.reciprocal_approx_fast(out=rep[:], in_=repw[:])
                    nc.vector.tensor_mul(
                        attn_oT[:, g * L + lh * 512: g * L + (lh + 1) * 512].bitcast(f32r),
                        avs[:], rep[:])

                units = [(hp, t) for hp in range(2) for t in range(8)]
                LAG = 4

                def make_avctx(g, lh):
                    state = {"drsc": None, "banks": None, "hp": None}

                    def process(item):
                        hp, t, ptc = item
                        if state["drsc"] is None:
                            state["drsc"] = dramp.tile([4, 64, 512], f32,
                                                       tag="drn", name="drsc")
                        if state["hp"] != hp:
                            state["banks"] = [
                                psp.tile([64, 512], f32, tag="ps", name=f"avh{jj}")
                                for jj in range(2)]
                            state["hp"] = hp
                        av_unit(state["banks"], item, g)
                        if t == 7:
                            copy_half(state["banks"], state["drsc"], hp)
                            if hp == 1:
                                finish_norm(state["drsc"], g, lh)
                    return process

                # --- block (g=0, lh=0): exps buffered, attn@V deferred ---
                # PE fillers paced against the ScalarE exp stream: co=1
                # projections early, conv_v quarters spread out.
                def q4_and_cc():
                    conv_quarter(pad_yf, wv_sb, vraw, 4, 1, 1, False)
                    emit_cc2_bn_v()

                fillers = {
                    1: lambda: proj_T(kraw, pk_sb, kT, 1),
                    3: lambda: proj_T(qraw, pq_sb, qT, 1),
                    5: lambda: conv_quarter(pad_yf, wv_sb, vraw, 4, 0, 1, False),
                    8: lambda: conv_quarter(pad_yf, wv_sb, vraw, 4, 1, 0, False),
                    11: q4_and_cc,
                }
                backlog = []
                for u, (hp, t) in enumerate(units):
                    backlog.append((hp, t, exp_unit(sc_unit(0, 0, t, hp))))
                    if u in fillers:
                        fillers[u]()

                def run_block(g, lh, pending, flush=None, bfillers=None):
                    # flush: deferred attn@V of the previous block; its
                    # matmuls interleave with this block's scores (starting
                    # only after the bfillers - e.g. the v projection they
                    # depend on - have been emitted) so the exp stream never
                    # drains.
                    proc = make_avctx(g, lh)
                    started = bfillers is None
                    for u, (hp, t) in enumerate(units[len(pending):]):
                        pending.append((hp, t, exp_unit(sc_unit(g, lh, t, hp))))
                        if bfillers and u in bfillers:
                            bfillers.pop(u)()
                            started = not bfillers
                            continue
                        if flush is not None and started:
                            fproc, fitems = flush
                            fproc(fitems.pop(0))
                            if fitems:
                                fproc(fitems.pop(0))
                            if not fitems:
                                flush = None
                        elif len(pending) > LAG and (flush is None):
                            proc(pending.pop(0))
                    if flush is not None:
                        fproc, fitems = flush
                        while fitems:
                            fproc(fitems.pop(0))
                    while pending:
                        proc(pending.pop(0))

                run_block(0, 1, [], flush=(make_avctx(0, 0), backlog),
                          bfillers={3: emit_vproj})

                # ---------- output projection (transposed) + bias ----------
                out_sb = singles.tile([128, 2 * L], f32)
                outr = out.rearrange("(c p) l -> p c l", p=128)
                osr = out_sb[:].rearrange("p (c l) -> p c l", l=L)

                def emit_outproj(lh):
                    for co in range(2):
                        ps = psp.tile([128, 512], f32, tag="ps", name="ops")
                        for ci in range(2):
                            lhsT = po_sb[:, ci * C + co * 128: ci * C + (co + 1) * 128]
                            rhs = attn_oT[:, ci * L + lh * 512: ci * L + (lh + 1) * 512].bitcast(f32r)
                            nc.tensor.matmul(ps[:], lhsT, rhs, start=(ci == 0), stop=(ci == 1))
                        nc.vector.tensor_scalar(
                            out=out_sb[:, co * L + lh * 512: co * L + (lh + 1) * 512],
                            in0=ps[:], scalar1=bot[:, co: co + 1], scalar2=None,
                            op0=ALU.add)
                    nc.sync.dma_start(out=outr[:, :, lh * 512:(lh + 1) * 512],
                                      in_=osr[:, :, lh * 512:(lh + 1) * 512])

                run_block(1, 0, [])
                emit_outproj(0)
                run_block(1, 1, [])
                emit_outproj(1)
                if dbg:
                    nc.sync.dma_start(out=dbg["dqT"].rearrange("p f -> p f"), in_=qT[:])
                    nc.sync.dma_start(out=dbg["dkT"].rearrange("p f -> p f"), in_=kT[:])
                    nc.sync.dma_start(out=dbg["dvsb"].rearrange("p f -> p f"), in_=v_sb[:])
                    nc.sync.dma_start(out=dbg["daoT"].rearrange("p f -> p f"), in_=attn_oT[:])

    nc.compile()
    return nc


def _prep_weights(conv_q_w, conv_k_w, conv_v_w, Wq, Wk, Wv, Wo,
                  bn_q_g, bn_q_b, bn_k_g, bn_k_b, bn_v_g, bn_v_b, bo):
    if CONV_FP8:
        from concourse import mybir
        np_fp8 = mybir.dt.np(mybir.dt.float8e4)

    def conv_tiles(w, to8=False):
        # [co, ci, ky, kx] -> [9, 2(ci), 2(co), 128, 128]
        t = np.ascontiguousarray(np.transpose(np.asarray(w, np.float32), (2, 3, 1, 0)))
        t = t.reshape(3, 3, 2, 128, 2, 128).transpose(0, 1, 2, 4, 3, 5)
        t = np.ascontiguousarray(t.reshape(9, 2, 2, 128, 128))
        if to8:
            t = (t * W8_SCALE).astype(np_fp8)
        return t

    def proj_tiles(w):
        return np.ascontiguousarray(
            np.asarray(w, np.float32).T.reshape(2, 128, C))

    gbp = np.zeros((128, 12), np.float32)
    for i, (g, b) in enumerate(((bn_q_g, bn_q_b), (bn_k_g, bn_k_b), (bn_v_g, bn_v_b))):
        g = np.asarray(g, np.float32).reshape(2, 128)
        b = np.asarray(b, np.float32).reshape(2, 128)
        for ch in range(2):
            gbp[:, 2 * i + ch] = g[ch]
            gbp[:, 6 + 2 * i + ch] = b[ch]
    bop = np.ascontiguousarray(np.asarray(bo, np.float32).reshape(2, 128).T)
    return {
        "wcq": conv_tiles(conv_q_w, CONV_FP8), "wck": conv_tiles(conv_k_w, CONV_FP8),
        "wcv": conv_tiles(conv_v_w),
        "pq": proj_tiles(Wq), "pk": proj_tiles(Wk), "pv": proj_tiles(Wv),
        "po": proj_tiles(Wo),
        "gb": gbp, "bo": bop,
    }


def _get_nc(repeat=1):
    key = ("nc", repeat, VARIANT, DEBUG, RSTD_MODE, DVE_EXP_EVERY, CONV_FP8)
    if key not in _CACHE:
        _CACHE[key] = _build_nc(repeat)
    return _CACHE[key]


def run_spmd(in_maps, repeat=1, **kw):
    from concourse.bass_utils import run_bass_kernel_spmd
    return run_bass_kernel_spmd(_get_nc(repeat), in_maps, list(range(8)), **kw)


def _get_executor(repeat=1):
    """Build the sharded jitted callable once so repeated calls skip
    retracing/compilation."""
    key = ("exec", repeat, VARIANT, RSTD_MODE, DVE_EXP_EVERY, CONV_FP8)
    if key in _CACHE:
        return _CACHE[key]
    import jax
    import numpy as _np
    from jax.sharding import Mesh, PartitionSpec
    from jax.experimental.shard_map import shard_map
    from concourse import bass2jax, mybir

    nc = _get_nc(repeat)
    bass2jax.install_neuronx_cc_hook()
    partition_name = nc.partition_id_tensor.name if nc.partition_id_tensor else None

    in_names, out_names, out_avals, zero_outs = [], [], [], []
    for alloc in nc.m.functions[0].allocations:
        if not isinstance(alloc, mybir.MemoryLocationSet):
            continue
        name = alloc.memorylocations[0].name
        if alloc.kind == "ExternalInput":
            if name != partition_name:
                in_names.append(name)
        elif alloc.kind == "ExternalOutput":
            dt_np = mybir.dt.np(alloc.dtype)
            shape = tuple(alloc.tensor_shape)
            out_avals.append(jax.core.ShapedArray(shape, dt_np))
            out_names.append(name)
            zero_outs.append(_np.zeros(shape, dt_np))

    n_params = len(in_names)
    n_outs = len(out_names)
    all_in_names = list(in_names) + list(out_names)
    if partition_name is not None:
        all_in_names.append(partition_name)
    donate = tuple(range(n_params, n_params + n_outs))

    def _body(*args):
        operands = list(args)
        if partition_name is not None:
            operands.append(bass2jax.partition_id_tensor())
        outs = bass2jax._bass_exec_p.bind(
            *operands,
            out_avals=tuple(out_avals),
            in_names=tuple(all_in_names),
            out_names=tuple(out_names),
            lowering_input_output_aliases=(),
            sim_require_finite=True,
            sim_require_nnan=True,
            nc=nc,
        )
        return tuple(outs)

    devices = jax.devices()[:B]
    mesh = Mesh(np.asarray(devices), ("core",))
    in_specs = (PartitionSpec("core"),) * (n_params + n_outs)
    out_specs = (PartitionSpec("core"),) * n_outs
    sharded = jax.jit(
        shard_map(_body, mesh=mesh, in_specs=in_specs, out_specs=out_specs,
                  check_rep=False),
        donate_argnums=donate, keep_unused=True,
    )
    _CACHE[("mesh", repeat, VARIANT)] = mesh
    _CACHE[("jit", repeat, VARIANT)] = sharded

    def run(in_maps):
        concat_in = [
            np.concatenate([np.asarray(in_maps[c][k]) for c in range(B)], axis=0)
            for k in in_names
        ]
        concat_zeros = [np.zeros((B * z.shape[0], *z.shape[1:]), z.dtype)
                        for z in zero_outs]
        out_arrs = sharded(*concat_in, *concat_zeros)
        return out_arrs, out_names, out_avals

    _CACHE[key] = run
    return run


def run_fast(in_maps, repeat=1):
    """Execute via the cached jitted callable; returns per-core dict list."""
    run = _get_executor(repeat)
    out_arrs, out_names, out_avals = run(in_maps)
    return [
        {name: np.asarray(out_arrs[i]).reshape(B, *out_avals[i].shape)[c]
         for i, name in enumerate(out_names)}
        for c in range(B)
    ]


def bench_wall(in_maps, repeat, n_iter):
    """Dispatch n_iter executions of the repeat-R NEFF with device-resident
    inputs and pre-staged donated zero buffers; return total wall seconds."""
    import time as _time
    import jax
    from jax.sharding import NamedSharding, PartitionSpec

    _get_executor(repeat)  # ensure built
    nc = _get_nc(repeat)
    from concourse import mybir
    partition_name = nc.partition_id_tensor.name if nc.partition_id_tensor else None
    in_names, out_shapes = [], []
    for alloc in nc.m.functions[0].allocations:
        if not isinstance(alloc, mybir.MemoryLocationSet):
            continue
        name = alloc.memorylocations[0].name
        if alloc.kind == "ExternalInput" and name != partition_name:
            in_names.append(name)
        elif alloc.kind == "ExternalOutput":
            out_shapes.append((tuple(alloc.tensor_shape), mybir.dt.np(alloc.dtype)))

    key = ("bench_in", repeat, VARIANT, CONV_FP8)
    if key not in _CACHE:
        mesh = _CACHE[("mesh", repeat, VARIANT)]
        sh = NamedSharding(mesh, PartitionSpec("core"))
        dev_in = [
            jax.device_put(
                np.concatenate([np.asarray(in_maps[c][k]) for c in range(B)], 0), sh)
            for k in in_names
        ]
        _CACHE[key] = (dev_in, sh)
    dev_in, sh = _CACHE[key]

    sharded = _CACHE[("jit", repeat, VARIANT)]
    zero_sets = []
    for _ in range(n_iter):
        zs = [jax.device_put(np.zeros((B * s[0], *s[1:]), dt), sh)
              for (s, dt) in out_shapes]
        zero_sets.append(zs)
    for zs in zero_sets:
        for z in zs:
            z.block_until_ready()

    outs = []
    t0 = _time.perf_counter()
    for it in range(n_iter):
        outs.append(sharded(*dev_in, *zero_sets[it]))
    for o in outs[-1]:
        o.block_until_ready()
    t1 = _time.perf_counter()
    return t1 - t0


def make_in_maps(x, y, h, w, conv_q_w, bn_q_g, bn_q_b,
                 conv_k_w, bn_k_g, bn_k_b, conv_v_w, bn_v_g, bn_v_b,
                 Wq, Wk, Wv, Wo, bo):
    assert int(h) == IMG and int(w) == IMG
    x = np.asarray(x, np.float32)
    y = np.asarray(y, np.float32)
    wmap = _prep_weights(conv_q_w, conv_k_w, conv_v_w, Wq, Wk, Wv, Wo,
                         bn_q_g, bn_q_b, bn_k_g, bn_k_b, bn_v_g, bn_v_b, bo)

    def pad_t(a, wpad, dtype):
        at = np.transpose(a, (0, 2, 1)).reshape(B, C, IMG, IMG)
        ap = np.zeros((B, C, PAD, wpad), np.float32)
        ap[:, :, 1:33, 1:33] = at
        return ap.reshape(B, C, PAD * wpad).astype(dtype)

    yTf = pad_t(y, PAD, np.float32)
    if CONV_FP8:
        from concourse import mybir
        np_fp8 = mybir.dt.np(mybir.dt.float8e4)
        xT = pad_t(x, PADW8, np_fp8)
        yT8 = pad_t(y, PADW8, np_fp8)
        return [dict(wmap, xt=xT[b], yt8=yT8[b], ytf=yTf[b]) for b in range(B)]
    xT = pad_t(x, PAD, np.float32)
    return [dict(wmap, xt=xT[b], ytf=yTf[b]) for b in range(B)]


def kernel(**inputs):
    in_maps = make_in_maps(**inputs)
    res = run_fast(in_maps)
    outs = [res[b]["out"] for b in range(B)]  # each [C, L]
    return np.ascontiguousarray(
        np.stack(outs, axis=0).transpose(0, 2, 1)).astype(np.float32)
